# revision 21
# baseline (speedup 1.0000x reference)
"""Trainium2 Bass kernel for nn_FMNet pixel-shuffle + sigmoid.

reference:  x = FV[:, 64:, :, :]                                 # [B, 64, 64, 64]
            out[b, 8i+r, 8j+c'] = sigmoid(x[b, 8r+c', i, j])     # [B, 1, 512, 512]

Per core (4 batches, pure data-parallel over batch):

Layout: partition p = (b:4, i2:32), i = 2*i2 + ip.  tin free =
(c:64, ip:2, j:64); tout free = (ip:2, r:8, q:512), q = 8j + c'.
Output row = 16*i2 + 8*ip + 2*cq + rp where channel c = 8r + c',
r = 2*cq + rp (cq = channel quarter).

HWDGE SDMA-lane use scales with a DMA's partition span (8-partition
DMAs run on half the lanes at twice the per-lane load), so every DMA
here spans 32 partitions, like the proven baseline shape:

  - loads: 16 DMAs of 256 KiB per (b, cq) - channel quarter cq gives
    512 B chunks (i-row pairs).  Issued quarter-major on two DGE
    streams (Sync: b0/b1, GpSimd SWDGE: b2/b3), so quarter 0 is
    resident early and compute pipelines behind the load stream.
  - compute: 4 ScalarE ACTIVATE(Sigmoid) [128 x 2048] per quarter
    (ACT duration measured ~2.0 us regardless of 1024 vs 2048 elems,
    so fewer/bigger ACTs halve the serial Scalar chain), strided-read
    interleave (c', j) -> q = j*8+c'.
  - stores: 16 DMAs of 256 KiB per (b, cq) with 4 KiB chunks (row
    pairs 2cq, 2cq+1); issued as soon as ACT(cq) retires, so stores
    overlap the remaining load waves.  Stores carry no semaphore and
    nothing waits on them: the NEFF has no reader of OUT, the host
    syncs on execution end, and the store tail drains during the
    fixed walrus epilogue (no_gpsimd_drain skips the SWDGE drain).

32 total DMA instructions (~0.65 us descriptor-gen each) split 16/16
across Sync/GpSimd keeps both queues fed; ScalarE only computes.
Separate semaphores per DGE kind (HWDGE vs SWDGE must not share).
"""

import os
import sys

if "/opt/trn_rl_repo" not in sys.path:
    sys.path.insert(0, "/opt/trn_rl_repo")

import numpy as np

import concourse.bass as bass
from concourse import mybir
from concourse.bass_utils import run_bass_kernel_spmd

N_CORES = 8
B = 32
B_LOC = B // N_CORES   # 4
H = W = 512
S = 64

LAST_EXEC_NS = None

_cached_nc = None


def _install_trace_hook():
    """Best-effort NTFF hook so BASS_TRACE=1 yields exec_time_ns."""
    try:
        import types

        import antenv

        try:
            from antenv.axon_hooks import get_axon_ntff_profile_hook  # noqa: F401

            return
        except ImportError:
            pass
        mod = types.ModuleType("antenv.axon_hooks")
        _state = {"hook": None}
        mod.set_axon_ntff_profile_hook = lambda h: _state.__setitem__("hook", h)
        mod.get_axon_ntff_profile_hook = lambda: _state["hook"]
        sys.modules["antenv.axon_hooks"] = mod
        antenv.axon_hooks = mod
        from trn_agent_boot.trn_boot import _ntff_profile_via_ctypes

        mod.set_axon_ntff_profile_hook(
            _ntff_profile_via_ctypes("/opt/axon/libaxon_pjrt.so")
        )
    except Exception:
        pass


def _build_nc():
    import contextlib

    F32 = mybir.dt.float32
    nc = bass.Bass("TRN2", num_devices=N_CORES)
    FV = nc.declare_dram_parameter("FV", [B_LOC, 128, S, S], F32, isOutput=False)
    OUT = nc.declare_dram_parameter("OUT", [B_LOC, W, H], F32, isOutput=True)

    tin = nc.alloc_sbuf_tensor("tin", [128, 8192], F32)
    tout = nc.alloc_sbuf_tensor("tout", [128, 8192], F32)
    scratch = nc.alloc_sbuf_tensor("scratch", [1, 8], F32)

    fv = FV[:]
    out = OUT[:]

    def load_aps(b, cq):
        """256 KiB load of channel quarter cq of batch b; [32 p] span."""
        ch0 = 64 + 16 * cq
        src = fv[b, ch0 : ch0 + 16].rearrange("c (i2 ip) j -> i2 c (ip j)", ip=2)
        dst = tin.ap()[32 * b : 32 * b + 32, 2048 * cq : 2048 * (cq + 1)].rearrange(
            "p (c v) -> p c v", c=16
        )
        return dst, src

    def load_oct_aps(b, g):
        """128 KiB load of channel octant g of batch b; [32 p] span."""
        ch0 = 64 + 8 * g
        src = fv[b, ch0 : ch0 + 8].rearrange("c (i2 ip) j -> i2 c (ip j)", ip=2)
        dst = tin.ap()[32 * b : 32 * b + 32, 1024 * g : 1024 * (g + 1)].rearrange(
            "p (c v) -> p c v", c=8
        )
        return dst, src

    def act_oct_aps(g):
        """Single-octant ACT [128, 2, 64, 8] (ip, j, c'): rows r = g."""
        in_v = tin.ap().rearrange(
            "p (c ip j) -> p c ip j", c=64, ip=2
        )[:, 8 * g : 8 * g + 8].rearrange("p c ip j -> p ip j c")
        out_v = tout.ap().rearrange(
            "p (ip r j c) -> p ip r j c", ip=2, r=8, j=64
        )[:, :, g]
        return out_v, in_v

    def act_aps(cq, ip):
        """ACT slice (channel quarter cq, row parity ip): [128, 2, 64, 8]."""
        in_v = tin.ap().rearrange(
            "p (rq rp cp ip j) -> p rq ip rp j cp", rq=4, rp=2, cp=8, ip=2
        )[:, cq, ip]
        out_v = tout.ap().rearrange(
            "p (ip rq rp j cp) -> p rq ip rp j cp", ip=2, rq=4, rp=2, j=64
        )[:, cq, ip]
        return out_v, in_v

    def store_aps(b, cq):
        """256 KiB store of rows 16*i2 + 8*ip + 2*cq + {0,1} of batch b."""
        dst = out[b].rearrange(
            "(i2 ip rq rp) q -> i2 ip rq (rp q)", i2=32, ip=2, rq=4
        )[:, :, cq, :]  # [32, 2, 1024]
        src = tout.ap()[32 * b : 32 * b + 32, :].rearrange(
            "p (ip rq v) -> p ip rq v", ip=2, rq=4
        )[:, :, cq, :]  # [32, 2, 1024]
        return dst, src

    with contextlib.ExitStack() as stack:
        # no_gpsimd_drain: skip the expensive SWDGE dge_drain at block exit;
        # the store stream drains on its own while the fixed walrus epilogue
        # (semaphore resets) runs.  No reader of OUT exists inside the NEFF,
        # and the host consumes outputs well after execution completes.
        block = stack.enter_context(nc.Block(no_gpsimd_drain=True))
        # HWDGE (Sync) and SWDGE (GpSimd) streams must not share semaphores.
        sem_lh = [stack.enter_context(nc.semaphore(f"sem_lh{q}")) for q in range(5)]
        sem_ls = [stack.enter_context(nc.semaphore(f"sem_ls{q}")) for q in range(5)]
        sem_act = stack.enter_context(nc.semaphore("sem_act"))
        sem_oh = stack.enter_context(nc.semaphore("sem_oh"))
        sem_os = stack.enter_context(nc.semaphore("sem_os"))

        @block.sync
        def _(sync: bass.BassEngine):
            for cq in range(3):
                for b in (0, 1):
                    dst, src = load_aps(b, cq)
                    sync.dma_start(out=dst, in_=src).then_inc(sem_lh[cq], 16)
            for w, g in ((3, 6), (4, 7)):
                for b in (0, 1):
                    dst, src = load_oct_aps(b, g)
                    sync.dma_start(out=dst, in_=src).then_inc(sem_lh[w], 16)
            for cq in range(4):
                sync.wait_ge(sem_act, max(2 * (cq + 1), 4))
                for b in (0, 1):
                    dst, src = store_aps(b, cq)
                    sync.dma_start(out=dst, in_=src).then_inc(sem_oh, 16)

        @block.gpsimd
        def _(g: bass.BassEngine):
            for cq in range(3):
                for b in (2, 3):
                    dst, src = load_aps(b, cq)
                    g.dma_start(out=dst, in_=src).then_inc(sem_ls[cq], 16)
            for w, gg in ((3, 6), (4, 7)):
                for b in (2, 3):
                    dst, src = load_oct_aps(b, gg)
                    g.dma_start(out=dst, in_=src).then_inc(sem_ls[w], 16)
            for cq in range(4):
                g.wait_ge(sem_act, max(2 * (cq + 1), 4))
                for b in (2, 3):
                    dst, src = store_aps(b, cq)
                    g.dma_start(out=dst, in_=src).then_inc(sem_os, 16)

        @block.scalar
        def _(scalar: bass.BassEngine):
            # dummy op to pull ACT_TABLE_LOAD (sigmoid) off the critical path
            scalar.activation(
                scratch.ap()[0:1, 0:1],
                nc.const_aps.tensor(0.0, (1, 1), mybir.dt.float32),
                mybir.ActivationFunctionType.Sigmoid,
            )
            for cq in range(3):
                scalar.wait_ge(sem_lh[cq], 2 * 16)
                scalar.wait_ge(sem_ls[cq], 2 * 16)
                for ip in (0, 1):
                    out_v, in_v = act_aps(cq, ip)
                    scalar.activation(
                        out_v, in_v, mybir.ActivationFunctionType.Sigmoid
                    ).then_inc(sem_act, 1)
            for w, gg in ((3, 6), (4, 7)):
                scalar.wait_ge(sem_lh[w], 2 * 16)
                scalar.wait_ge(sem_ls[w], 2 * 16)
                out_v, in_v = act_oct_aps(gg)
                scalar.activation(
                    out_v, in_v, mybir.ActivationFunctionType.Sigmoid
                ).then_inc(sem_act, 1)

    return nc


def kernel(FV, batch_size=None, W=None, H=None, **_ignored):
    global _cached_nc, LAST_EXEC_NS
    FV = np.asarray(FV, dtype=np.float32)
    assert FV.shape == (B, 128, S, S), FV.shape

    trace = bool(os.environ.get("BASS_TRACE"))
    if trace:
        _install_trace_hook()

    if _cached_nc is None:
        _cached_nc = _build_nc()
    nc = _cached_nc

    in_maps = [{"FV": FV[k * B_LOC : (k + 1) * B_LOC]} for k in range(N_CORES)]
    res = None
    for attempt in range(3):
        try:
            res = run_bass_kernel_spmd(nc, in_maps, list(range(N_CORES)), trace=trace)
            break
        except Exception:
            # occasional transient NRT_EXEC_UNIT_UNRECOVERABLE on a cold
            # device; retry after a short pause
            if attempt == 2:
                raise
            import time

            time.sleep(2.0)
    if trace:
        LAST_EXEC_NS = res.exec_time_ns

    outs = [res.results[k]["OUT"] for k in range(N_CORES)]
    full = np.concatenate(outs, axis=0)  # [32, 512, 512]
    return full[:, None, :, :].astype(np.float32)


# revision 23
# speedup vs baseline: 1.0093x; 1.0093x over previous
"""Trainium2 Bass kernel for nn_FMNet pixel-shuffle + sigmoid.

reference:  x = FV[:, 64:, :, :]                                 # [B, 64, 64, 64]
            out[b, 8i+r, 8j+c'] = sigmoid(x[b, 8r+c', i, j])     # [B, 1, 512, 512]

Per core (4 batches, pure data-parallel over batch):

Layout: partition p = (b:4, i2:32), i = 2*i2 + ip.  tin free =
(c:64, ip:2, j:64); tout free = (ip:2, r:8, q:512), q = 8j + c'.
Output row = 16*i2 + 8*ip + 2*cq + rp where channel c = 8r + c',
r = 2*cq + rp (cq = channel quarter).

HWDGE SDMA-lane use scales with a DMA's partition span (8-partition
DMAs run on half the lanes at twice the per-lane load), so every DMA
here spans 32 partitions, like the proven baseline shape:

  - loads: 16 DMAs of 256 KiB per (b, cq) - channel quarter cq gives
    512 B chunks (i-row pairs).  Issued quarter-major on two DGE
    streams (Sync: b0/b1, GpSimd SWDGE: b2/b3), so quarter 0 is
    resident early and compute pipelines behind the load stream.
  - compute: 4 ScalarE ACTIVATE(Sigmoid) [128 x 2048] per quarter
    (ACT duration measured ~2.0 us regardless of 1024 vs 2048 elems,
    so fewer/bigger ACTs halve the serial Scalar chain), strided-read
    interleave (c', j) -> q = j*8+c'.
  - stores: 16 DMAs of 256 KiB per (b, cq) with 4 KiB chunks (row
    pairs 2cq, 2cq+1); issued as soon as ACT(cq) retires, so stores
    overlap the remaining load waves.  Stores carry no semaphore and
    nothing waits on them: the NEFF has no reader of OUT, the host
    syncs on execution end, and the store tail drains during the
    fixed walrus epilogue (no_gpsimd_drain skips the SWDGE drain).

32 total DMA instructions (~0.65 us descriptor-gen each) split 16/16
across Sync/GpSimd keeps both queues fed; ScalarE only computes.
Separate semaphores per DGE kind (HWDGE vs SWDGE must not share).
"""

import os
import sys

if "/opt/trn_rl_repo" not in sys.path:
    sys.path.insert(0, "/opt/trn_rl_repo")

import numpy as np

import concourse.bass as bass
from concourse import mybir
from concourse.bass_utils import run_bass_kernel_spmd

N_CORES = 8
B = 32
B_LOC = B // N_CORES   # 4
H = W = 512
S = 64

LAST_EXEC_NS = None

_cached_nc = None


def _install_trace_hook():
    """Best-effort NTFF hook so BASS_TRACE=1 yields exec_time_ns."""
    try:
        import types

        import antenv

        try:
            from antenv.axon_hooks import get_axon_ntff_profile_hook  # noqa: F401

            return
        except ImportError:
            pass
        mod = types.ModuleType("antenv.axon_hooks")
        _state = {"hook": None}
        mod.set_axon_ntff_profile_hook = lambda h: _state.__setitem__("hook", h)
        mod.get_axon_ntff_profile_hook = lambda: _state["hook"]
        sys.modules["antenv.axon_hooks"] = mod
        antenv.axon_hooks = mod
        from trn_agent_boot.trn_boot import _ntff_profile_via_ctypes

        mod.set_axon_ntff_profile_hook(
            _ntff_profile_via_ctypes("/opt/axon/libaxon_pjrt.so")
        )
    except Exception:
        pass


def _build_nc():
    import contextlib

    F32 = mybir.dt.float32
    nc = bass.Bass("TRN2", num_devices=N_CORES)
    FV = nc.declare_dram_parameter("FV", [B_LOC, 128, S, S], F32, isOutput=False)
    OUT = nc.declare_dram_parameter("OUT", [B_LOC, W, H], F32, isOutput=True)

    tin = nc.alloc_sbuf_tensor("tin", [128, 8192], F32)
    tout = nc.alloc_sbuf_tensor("tout", [128, 8192], F32)
    scratch = nc.alloc_sbuf_tensor("scratch", [1, 8], F32)

    fv = FV[:]
    out = OUT[:]

    def load_aps(b, cq):
        """256 KiB load of channel quarter cq of batch b; [32 p] span."""
        ch0 = 64 + 16 * cq
        src = fv[b, ch0 : ch0 + 16].rearrange("c (i2 ip) j -> i2 c (ip j)", ip=2)
        dst = tin.ap()[32 * b : 32 * b + 32, 2048 * cq : 2048 * (cq + 1)].rearrange(
            "p (c v) -> p c v", c=16
        )
        return dst, src

    def act_aps(cq, ip):
        """ACT slice (channel quarter cq, row parity ip): [128, 2, 64, 8]."""
        in_v = tin.ap().rearrange(
            "p (rq rp cp ip j) -> p rq ip rp j cp", rq=4, rp=2, cp=8, ip=2
        )[:, cq, ip]
        out_v = tout.ap().rearrange(
            "p (ip rq rp j cp) -> p rq ip rp j cp", ip=2, rq=4, rp=2, j=64
        )[:, cq, ip]
        return out_v, in_v

    def store_aps(b, cq):
        """256 KiB store of rows 16*i2 + 8*ip + 2*cq + {0,1} of batch b."""
        dst = out[b].rearrange(
            "(i2 ip rq rp) q -> i2 ip rq (rp q)", i2=32, ip=2, rq=4
        )[:, :, cq, :]  # [32, 2, 1024]
        src = tout.ap()[32 * b : 32 * b + 32, :].rearrange(
            "p (ip rq v) -> p ip rq v", ip=2, rq=4
        )[:, :, cq, :]  # [32, 2, 1024]
        return dst, src

    with contextlib.ExitStack() as stack:
        # no_gpsimd_drain: skip the expensive SWDGE dge_drain at block exit;
        # the store stream drains on its own while the fixed walrus epilogue
        # (semaphore resets) runs.  No reader of OUT exists inside the NEFF,
        # and the host consumes outputs well after execution completes.
        block = stack.enter_context(nc.Block(no_gpsimd_drain=True))
        # HWDGE (Sync) and SWDGE (GpSimd) streams must not share semaphores.
        sem_lh = [stack.enter_context(nc.semaphore(f"sem_lh{q}")) for q in range(4)]
        sem_ls = [stack.enter_context(nc.semaphore(f"sem_ls{q}")) for q in range(4)]
        sem_act = stack.enter_context(nc.semaphore("sem_act"))
        sem_oh = stack.enter_context(nc.semaphore("sem_oh"))
        sem_os = stack.enter_context(nc.semaphore("sem_os"))

        @block.sync
        def _(sync: bass.BassEngine):
            for cq in range(4):
                for b in (0, 1):
                    dst, src = load_aps(b, cq)
                    sync.dma_start(out=dst, in_=src).then_inc(sem_lh[cq], 16)
            for cq in range(4):
                sync.wait_ge(sem_act, 2 * (cq + 1))
                for b in (0, 1):
                    dst, src = store_aps(b, cq)
                    sync.dma_start(out=dst, in_=src).then_inc(sem_oh, 16)

        @block.gpsimd
        def _(g: bass.BassEngine):
            for cq in range(4):
                for b in (2, 3):
                    dst, src = load_aps(b, cq)
                    g.dma_start(out=dst, in_=src).then_inc(sem_ls[cq], 16)
            for cq in range(4):
                g.wait_ge(sem_act, 2 * (cq + 1))
                for b in (2, 3):
                    dst, src = store_aps(b, cq)
                    g.dma_start(out=dst, in_=src).then_inc(sem_os, 16)

        @block.scalar
        def _(scalar: bass.BassEngine):
            # dummy op to pull ACT_TABLE_LOAD (sigmoid) off the critical path
            scalar.activation(
                scratch.ap()[0:1, 0:1],
                nc.const_aps.tensor(0.0, (1, 1), mybir.dt.float32),
                mybir.ActivationFunctionType.Sigmoid,
            )
            for cq in range(4):
                scalar.wait_ge(sem_lh[cq], 2 * 16)
                scalar.wait_ge(sem_ls[cq], 2 * 16)
                for ip in (0, 1):
                    out_v, in_v = act_aps(cq, ip)
                    scalar.activation(
                        out_v, in_v, mybir.ActivationFunctionType.Sigmoid
                    ).then_inc(sem_act, 1)

    return nc


def kernel(FV, batch_size=None, W=None, H=None, **_ignored):
    global _cached_nc, LAST_EXEC_NS
    FV = np.asarray(FV, dtype=np.float32)
    assert FV.shape == (B, 128, S, S), FV.shape

    trace = bool(os.environ.get("BASS_TRACE"))
    if trace:
        _install_trace_hook()

    if _cached_nc is None:
        _cached_nc = _build_nc()
    nc = _cached_nc

    in_maps = [{"FV": FV[k * B_LOC : (k + 1) * B_LOC]} for k in range(N_CORES)]
    res = None
    for attempt in range(3):
        try:
            res = run_bass_kernel_spmd(nc, in_maps, list(range(N_CORES)), trace=trace)
            break
        except Exception:
            # occasional transient NRT_EXEC_UNIT_UNRECOVERABLE on a cold
            # device; retry after a short pause
            if attempt == 2:
                raise
            import time

            time.sleep(2.0)
    if trace:
        LAST_EXEC_NS = res.exec_time_ns

    outs = [res.results[k]["OUT"] for k in range(N_CORES)]
    full = np.concatenate(outs, axis=0)  # [32, 512, 512]
    return full[:, None, :, :].astype(np.float32)


# revision 24
# speedup vs baseline: 1.0351x; 1.0256x over previous
"""Trainium2 Bass kernel for nn_FMNet pixel-shuffle + sigmoid.

reference:  x = FV[:, 64:, :, :]                                 # [B, 64, 64, 64]
            out[b, 8i+r, 8j+c'] = sigmoid(x[b, 8r+c', i, j])     # [B, 1, 512, 512]

Per core (4 batches, pure data-parallel over batch):

Layout: partition p = (b:4, i2:32), i = 2*i2 + ip.  tin free =
(c:64, ip:2, j:64); tout free = (ip:2, r:8, q:512), q = 8j + c'.
Output row = 16*i2 + 8*ip + 2*cq + rp where channel c = 8r + c',
r = 2*cq + rp (cq = channel quarter).

HWDGE SDMA-lane use scales with a DMA's partition span (8-partition
DMAs run on half the lanes at twice the per-lane load), so every DMA
here spans 32 partitions, like the proven baseline shape:

  - loads: 16 DMAs of 256 KiB per (b, cq) - channel quarter cq gives
    512 B chunks (i-row pairs).  Issued quarter-major on two DGE
    streams (Sync: b0/b1, GpSimd SWDGE: b2/b3), so quarter 0 is
    resident early and compute pipelines behind the load stream.
  - compute: 4 ScalarE ACTIVATE(Sigmoid) [128 x 2048] per quarter
    (ACT duration measured ~2.0 us regardless of 1024 vs 2048 elems,
    so fewer/bigger ACTs halve the serial Scalar chain), strided-read
    interleave (c', j) -> q = j*8+c'.
  - stores: 16 DMAs of 256 KiB per (b, cq) with 4 KiB chunks (row
    pairs 2cq, 2cq+1); issued as soon as ACT(cq) retires, so stores
    overlap the remaining load waves.  Stores carry no semaphore and
    nothing waits on them: the NEFF has no reader of OUT, the host
    syncs on execution end, and the store tail drains during the
    fixed walrus epilogue (no_gpsimd_drain skips the SWDGE drain).

32 total DMA instructions (~0.65 us descriptor-gen each) split 16/16
across Sync/GpSimd keeps both queues fed; ScalarE only computes.
Separate semaphores per DGE kind (HWDGE vs SWDGE must not share).
"""

import os
import sys

if "/opt/trn_rl_repo" not in sys.path:
    sys.path.insert(0, "/opt/trn_rl_repo")

import numpy as np

import concourse.bass as bass
from concourse import mybir
from concourse.bass_utils import run_bass_kernel_spmd

N_CORES = 8
B = 32
B_LOC = B // N_CORES   # 4
H = W = 512
S = 64

LAST_EXEC_NS = None

_cached_nc = None


def _install_trace_hook():
    """Best-effort NTFF hook so BASS_TRACE=1 yields exec_time_ns."""
    try:
        import types

        import antenv

        try:
            from antenv.axon_hooks import get_axon_ntff_profile_hook  # noqa: F401

            return
        except ImportError:
            pass
        mod = types.ModuleType("antenv.axon_hooks")
        _state = {"hook": None}
        mod.set_axon_ntff_profile_hook = lambda h: _state.__setitem__("hook", h)
        mod.get_axon_ntff_profile_hook = lambda: _state["hook"]
        sys.modules["antenv.axon_hooks"] = mod
        antenv.axon_hooks = mod
        from trn_agent_boot.trn_boot import _ntff_profile_via_ctypes

        mod.set_axon_ntff_profile_hook(
            _ntff_profile_via_ctypes("/opt/axon/libaxon_pjrt.so")
        )
    except Exception:
        pass


def _build_nc():
    import contextlib

    F32 = mybir.dt.float32
    nc = bass.Bass("TRN2", num_devices=N_CORES)
    FV = nc.declare_dram_parameter("FV", [B_LOC, 128, S, S], F32, isOutput=False)
    OUT = nc.declare_dram_parameter("OUT", [B_LOC, W, H], F32, isOutput=True)

    tin = nc.alloc_sbuf_tensor("tin", [128, 8192], F32)
    tout = nc.alloc_sbuf_tensor("tout", [128, 8192], F32)
    scratch = nc.alloc_sbuf_tensor("scratch", [1, 8], F32)

    fv = FV[:]
    out = OUT[:]

    def load_aps(b, cq):
        """256 KiB load of channel quarter cq of batch b; [32 p] span."""
        ch0 = 64 + 16 * cq
        src = fv[b, ch0 : ch0 + 16].rearrange("c (i2 ip) j -> i2 c (ip j)", ip=2)
        dst = tin.ap()[32 * b : 32 * b + 32, 2048 * cq : 2048 * (cq + 1)].rearrange(
            "p (c v) -> p c v", c=16
        )
        return dst, src

    def load_oct_aps(b, g):
        """128 KiB load of channel octant g of batch b; [32 p] span."""
        ch0 = 64 + 8 * g
        src = fv[b, ch0 : ch0 + 8].rearrange("c (i2 ip) j -> i2 c (ip j)", ip=2)
        dst = tin.ap()[32 * b : 32 * b + 32, 1024 * g : 1024 * (g + 1)].rearrange(
            "p (c v) -> p c v", c=8
        )
        return dst, src

    def act_oct_aps(g):
        """Single-octant ACT [128, 2, 64, 8] (ip, j, c'): rows r = g."""
        in_v = tin.ap().rearrange(
            "p (c ip j) -> p c ip j", c=64, ip=2
        )[:, 8 * g : 8 * g + 8].rearrange("p c ip j -> p ip j c")
        out_v = tout.ap().rearrange(
            "p (ip r j c) -> p ip r j c", ip=2, r=8, j=64
        )[:, :, g]
        return out_v, in_v

    def act_aps(cq, ip):
        """ACT slice (channel quarter cq, row parity ip): [128, 2, 64, 8]."""
        in_v = tin.ap().rearrange(
            "p (rq rp cp ip j) -> p rq ip rp j cp", rq=4, rp=2, cp=8, ip=2
        )[:, cq, ip]
        out_v = tout.ap().rearrange(
            "p (ip rq rp j cp) -> p rq ip rp j cp", ip=2, rq=4, rp=2, j=64
        )[:, cq, ip]
        return out_v, in_v

    def store_aps(b, cq):
        """256 KiB store of rows 16*i2 + 8*ip + 2*cq + {0,1} of batch b."""
        dst = out[b].rearrange(
            "(i2 ip rq rp) q -> i2 ip rq (rp q)", i2=32, ip=2, rq=4
        )[:, :, cq, :]  # [32, 2, 1024]
        src = tout.ap()[32 * b : 32 * b + 32, :].rearrange(
            "p (ip rq v) -> p ip rq v", ip=2, rq=4
        )[:, :, cq, :]  # [32, 2, 1024]
        return dst, src

    with contextlib.ExitStack() as stack:
        # no_gpsimd_drain: skip the expensive SWDGE dge_drain at block exit;
        # the store stream drains on its own while the fixed walrus epilogue
        # (semaphore resets) runs.  No reader of OUT exists inside the NEFF,
        # and the host consumes outputs well after execution completes.
        block = stack.enter_context(nc.Block(no_gpsimd_drain=True))
        # HWDGE (Sync) and SWDGE (GpSimd) streams must not share semaphores.
        sem_lh = [stack.enter_context(nc.semaphore(f"sem_lh{q}")) for q in range(6)]
        sem_ls = [stack.enter_context(nc.semaphore(f"sem_ls{q}")) for q in range(6)]
        sem_act = stack.enter_context(nc.semaphore("sem_act"))
        sem_oh = stack.enter_context(nc.semaphore("sem_oh"))
        sem_os = stack.enter_context(nc.semaphore("sem_os"))

        @block.sync
        def _(sync: bass.BassEngine):
            for cq in range(2):
                for b in (0, 1):
                    dst, src = load_aps(b, cq)
                    sync.dma_start(out=dst, in_=src).then_inc(sem_lh[cq], 16)
            for w, g in ((2, 4), (3, 5), (4, 6), (5, 7)):
                for b in (0, 1):
                    dst, src = load_oct_aps(b, g)
                    sync.dma_start(out=dst, in_=src).then_inc(sem_lh[w], 16)
            for cq in range(4):
                sync.wait_ge(sem_act, 2 * (cq + 1))
                for b in (0, 1):
                    dst, src = store_aps(b, cq)
                    sync.dma_start(out=dst, in_=src).then_inc(sem_oh, 16)

        @block.gpsimd
        def _(g: bass.BassEngine):
            for cq in range(2):
                for b in (2, 3):
                    dst, src = load_aps(b, cq)
                    g.dma_start(out=dst, in_=src).then_inc(sem_ls[cq], 16)
            for w, gg in ((2, 4), (3, 5), (4, 6), (5, 7)):
                for b in (2, 3):
                    dst, src = load_oct_aps(b, gg)
                    g.dma_start(out=dst, in_=src).then_inc(sem_ls[w], 16)
            for cq in range(4):
                g.wait_ge(sem_act, 2 * (cq + 1))
                for b in (2, 3):
                    dst, src = store_aps(b, cq)
                    g.dma_start(out=dst, in_=src).then_inc(sem_os, 16)

        @block.scalar
        def _(scalar: bass.BassEngine):
            # dummy op to pull ACT_TABLE_LOAD (sigmoid) off the critical path
            scalar.activation(
                scratch.ap()[0:1, 0:1],
                nc.const_aps.tensor(0.0, (1, 1), mybir.dt.float32),
                mybir.ActivationFunctionType.Sigmoid,
            )
            for cq in range(2):
                scalar.wait_ge(sem_lh[cq], 2 * 16)
                scalar.wait_ge(sem_ls[cq], 2 * 16)
                for ip in (0, 1):
                    out_v, in_v = act_aps(cq, ip)
                    scalar.activation(
                        out_v, in_v, mybir.ActivationFunctionType.Sigmoid
                    ).then_inc(sem_act, 1)
            for w, gg in ((2, 4), (3, 5), (4, 6), (5, 7)):
                scalar.wait_ge(sem_lh[w], 2 * 16)
                scalar.wait_ge(sem_ls[w], 2 * 16)
                out_v, in_v = act_oct_aps(gg)
                scalar.activation(
                    out_v, in_v, mybir.ActivationFunctionType.Sigmoid
                ).then_inc(sem_act, 1)

    return nc


def kernel(FV, batch_size=None, W=None, H=None, **_ignored):
    global _cached_nc, LAST_EXEC_NS
    FV = np.asarray(FV, dtype=np.float32)
    assert FV.shape == (B, 128, S, S), FV.shape

    trace = bool(os.environ.get("BASS_TRACE"))
    if trace:
        _install_trace_hook()

    if _cached_nc is None:
        _cached_nc = _build_nc()
    nc = _cached_nc

    in_maps = [{"FV": FV[k * B_LOC : (k + 1) * B_LOC]} for k in range(N_CORES)]
    res = None
    for attempt in range(3):
        try:
            res = run_bass_kernel_spmd(nc, in_maps, list(range(N_CORES)), trace=trace)
            break
        except Exception:
            # occasional transient NRT_EXEC_UNIT_UNRECOVERABLE on a cold
            # device; retry after a short pause
            if attempt == 2:
                raise
            import time

            time.sleep(2.0)
    if trace:
        LAST_EXEC_NS = res.exec_time_ns

    outs = [res.results[k]["OUT"] for k in range(N_CORES)]
    full = np.concatenate(outs, axis=0)  # [32, 512, 512]
    return full[:, None, :, :].astype(np.float32)
